# revision 2
# baseline (speedup 1.0000x reference)
"""Trainium2 Bass kernel for a 2-layer GAT (DGL-style) on a random graph.

v3 strategy (8 NeuronCores, SPMD, dst-block sharding):
  - 392 dst blocks of 128 nodes, LPT-balanced 49/core; blocks grouped
    (strided for balance) so each group needs only 2 dma_gathers (lo/hi).
  - Per-head rotation: W1rot = [W1 @ R | Vr1] with R_h = [Q_h | attn_l_h]
    (Q orthonormal basis of attn_l^perp), so el1 is coordinate 31 of each
    head's 32 and the L1 gather row is exactly 256 bf16 = 512B. A per-block
    rotate-back matmul (block-diag R^-1) restores features before relu.
  - Phase 1 (bf16): y1|er1 = x @ [W1R | Vr1]; er1 kept resident.
  - 4 split AllGathers overlap phase 1 (table is range-major).
  - Edge phase per group: 2 dma_gathers; per block: indicators via
    is_equal (int8 in / bf16 out), er via indicator-transpose matmuls,
    w = exp(leakyrelu(el+er)), one accumulating matmul per 128-edge chunk:
    psum += Ind^T @ [w*y | w]. Rotate-back + normalize + bias + relu -> h.
  - T2 rows [y2(40) | 1.0 | el2 | pad] bf16 (256B); L2 analogous with w
    folded into the 41-wide rhs; denominator from the baked 1.0 column.
"""

import sys
sys.path.insert(0, '/opt/trn_rl_repo')

import numpy as np
import ml_dtypes

N_NODES = 50000
N_EDGES = 800000
F_IN = 256
H1, HD = 8, 32
C2 = 40
NEG_SLOPE = 0.2
NCORES = 8
P = 128
BLOCKS_PER_CORE = 49
NODES_PER_CORE = BLOCKS_PER_CORE * P      # 6272
NPAD = NCORES * NODES_PER_CORE            # 50176
NBLOCKS = NPAD // P                       # 392
SPLIT = 32768
NGROUPS = 17
AGRANGES = [(0, 49)]

AGR_TAB = []
_off = 0
for (_a, _b) in AGRANGES:
    size = (_b - _a) * P * NCORES
    AGR_TAB.append((_off, _off + size))
    _off += size
assert _off == NPAD


def _prep_graph(src, dst):
    src = src.astype(np.int64)
    dst = dst.astype(np.int64)
    blk_of_edge = dst // P
    blk_counts = np.bincount(blk_of_edge, minlength=NBLOCKS)

    order = np.argsort(-blk_counts)
    core_of_blk = np.zeros(NBLOCKS, np.int64)
    loads = np.zeros(NCORES, np.int64)
    fills = np.zeros(NCORES, np.int64)
    for b in order:
        cands = np.where(fills < BLOCKS_PER_CORE)[0]
        c = cands[np.argmin(loads[cands])]
        core_of_blk[b] = c
        loads[c] += blk_counts[b]
        fills[c] += 1

    pos_of_blk = np.zeros(NBLOCKS, np.int64)
    blocks_at = np.zeros((NCORES, BLOCKS_PER_CORE), np.int64)
    for c in range(NCORES):
        mine = np.where(core_of_blk == c)[0]
        mine = mine[np.argsort(-blk_counts[mine])]
        blocks_at[c] = mine
        pos_of_blk[mine] = np.arange(BLOCKS_PER_CORE)

    # range-major table row-block of (core, pos) for split AllGathers
    rowblk_of_corepos = np.zeros((NCORES, BLOCKS_PER_CORE), np.int64)
    for c in range(NCORES):
        for p_ in range(BLOCKS_PER_CORE):
            for (a, b) in AGRANGES:
                if a <= p_ < b:
                    rowblk_of_corepos[c, p_] = a * NCORES + c * (b - a) + (p_ - a)
                    break

    node_ids = np.arange(NPAD)
    nb = node_ids // P
    pos_of_node = (rowblk_of_corepos[core_of_blk[nb], pos_of_blk[nb]] * P
                   + (node_ids % P))

    e_order = np.argsort(blk_of_edge, kind='stable')
    blk_starts = np.zeros(NBLOCKS + 1, np.int64)
    np.cumsum(blk_counts, out=blk_starts[1:])

    srcpos = pos_of_node[src]
    dstloc = (dst % P).astype(np.int64)

    groups = [list(range(gidx, BLOCKS_PER_CORE, NGROUPS)) for gidx in range(NGROUPS)]

    n_lo = np.zeros((NCORES, BLOCKS_PER_CORE), np.int64)
    n_hi = np.zeros((NCORES, BLOCKS_PER_CORE), np.int64)
    blk_lo_edges = {}
    blk_hi_edges = {}
    for b in range(NBLOCKS):
        es = e_order[blk_starts[b]:blk_starts[b + 1]]
        sp = srcpos[es]
        lo = es[sp < SPLIT]
        hi = es[sp >= SPLIT]
        lo = lo[np.argsort(srcpos[lo], kind='stable')]
        hi = hi[np.argsort(srcpos[hi], kind='stable')]
        blk_lo_edges[b] = lo
        blk_hi_edges[b] = hi
        c, i = core_of_blk[b], pos_of_blk[b]
        n_lo[c, i] = -(-len(lo) // P) if len(lo) else 0
        n_hi[c, i] = -(-len(hi) // P) if len(hi) else 0
    n_lo_max = n_lo.max(axis=0)
    n_hi_max = n_hi.max(axis=0)
    nb_tot = n_lo_max + n_hi_max
    return dict(core_of_blk=core_of_blk, pos_of_blk=pos_of_blk, blocks_at=blocks_at,
                pos_of_node=pos_of_node, blk_lo=blk_lo_edges, blk_hi=blk_hi_edges,
                srcpos=srcpos, dstloc=dstloc, n_lo_max=n_lo_max, n_hi_max=n_hi_max,
                nb_tot=nb_tot, groups=groups)


def _pack_idx16(vals):
    v = np.asarray(vals, np.uint16).reshape(-1, 16).T
    return np.tile(v, (8, 1)).view(np.int16)


def _rotations(W1, attn_l1, attn_r1):
    """Per-head R_h = [Q_h | a_h] (Q orthonormal perp of a); W1rot = W1 @ R."""
    rng = np.random.default_rng(0)
    W1 = W1.astype(np.float64)
    W1rot = np.zeros((F_IN, F_IN), np.float64)
    Rinv_bd = np.zeros((F_IN, F_IN), np.float64)
    for h in range(H1):
        a = attn_l1[h].astype(np.float64)
        M = np.concatenate([a[:, None], rng.standard_normal((HD, HD - 1))], axis=1)
        Q, _ = np.linalg.qr(M)
        Qp = Q[:, 1:]
        R = np.concatenate([Qp, a[:, None]], axis=1)
        aa = float(a @ a)
        Rinv = np.concatenate([Qp.T, (a / aa)[None, :]], axis=0)
        assert np.allclose(R @ Rinv, np.eye(HD), atol=1e-9)
        sl = slice(h * HD, (h + 1) * HD)
        W1rot[:, sl] = W1[:, sl] @ R
        Rinv_bd[sl, sl] = Rinv
    Vr1 = np.einsum('khd,hd->kh', W1.reshape(F_IN, H1, HD), attn_r1.astype(np.float64))
    Wcat1 = np.concatenate([W1rot, Vr1], axis=1)   # [256, 264]
    return Wcat1.astype(np.float32), Rinv_bd.astype(np.float32)


def _build_core_inputs(g, features, W1, attn_l1, attn_r1, W2, attn_l2, attn_r2,
                       b1, b2):
    bf = ml_dtypes.bfloat16
    CTOT = int(g['nb_tot'].sum())
    NBB = int(g['nb_tot'].max())
    gch = [int(sum(g['nb_tot'][i] for i in grp)) for grp in g['groups']]
    GMAX = max(gch)

    Wcat1, Rinv_bd = _rotations(W1, attn_l1, attn_r1)
    vl2 = W2 @ attn_l2[0]
    vr2 = W2 @ attn_r2[0]
    Wcat2 = np.concatenate([W2, vl2[:, None], vr2[:, None]], axis=1)  # [256,42]

    iotaR_i8 = np.tile(np.arange(P, dtype=np.int8)[None, :], (P, NBB))
    iotaC_i8 = np.tile(np.arange(P, dtype=np.int8)[:, None], (1, NBB * P))
    b1B = np.tile(b1.astype(np.float32)[None, :], (P, 1))
    b2B = np.tile(b2.astype(np.float32)[None, :], (P, 1))
    RinvT = np.zeros((P, 2, P), np.float32)
    RinvT[:, 0, :] = Rinv_bd[0:128, 0:128]
    RinvT[:, 1, :] = Rinv_bd[128:256, 128:256]

    feats_pad = np.zeros((NPAD, F_IN), np.float32)
    feats_pad[:N_NODES] = features

    per_core = []
    chunk_off = None
    for c in range(NCORES):
        my_nodes = (g['blocks_at'][c][:, None] * P + np.arange(P)[None, :]).reshape(-1)
        xT = feats_pad[my_nodes].T.astype(bf).copy()

        idx_cols = []
        dstloc_i8 = np.full((P, CTOT), -1, np.int8)
        dstlocT = np.full((1, CTOT * P), -1, np.int8)
        # gather-order (group, kind, block) for idx packing
        for grp in g['groups']:
            for kind in ('lo', 'hi'):
                for i in grp:
                    b = g['blocks_at'][c][i]
                    nch = int((g['n_lo_max'] if kind == 'lo' else g['n_hi_max'])[i])
                    if nch == 0:
                        continue
                    es = g['blk_lo' if kind == 'lo' else 'blk_hi'][b]
                    sp = g['srcpos'][es] - (0 if kind == 'lo' else SPLIT)
                    npad_e = nch * P - len(es)
                    sp = np.concatenate([sp, np.zeros(npad_e, np.int64)])
                    idx_cols.append(_pack_idx16(sp))
        # block-major (group order) for dstloc/dstlocT
        ccol = 0
        co = {}
        for grp in g['groups']:
            for i in grp:
                co[i] = ccol
                b = g['blocks_at'][c][i]
                for kind in ('lo', 'hi'):
                    nch = int((g['n_lo_max'] if kind == 'lo' else g['n_hi_max'])[i])
                    if nch == 0:
                        continue
                    es = g['blk_lo' if kind == 'lo' else 'blk_hi'][b]
                    dl = g['dstloc'][es]
                    npad_e = nch * P - len(es)
                    dl = np.concatenate([dl, np.full(npad_e, -1, np.int64)])
                    dstloc_i8[:, ccol:ccol + nch] = dl.astype(np.int8).reshape(nch, P).T
                    dstlocT[0, ccol * P:(ccol + nch) * P] = dl.astype(np.int8)
                    ccol += nch
        assert ccol == CTOT, (ccol, CTOT)
        chunk_off = co
        idx_all = np.concatenate(idx_cols, axis=1)

        per_core.append(dict(xT=xT, idx=idx_all, dstloc=dstloc_i8, dstlocT=dstlocT,
                             Wcat1=Wcat1.astype(bf), Wcat2=Wcat2.astype(bf),
                             RinvT=RinvT.astype(bf),
                             iotaR=iotaR_i8, iotaC=iotaC_i8, b1B=b1B, b2B=b2B))
    return per_core, CTOT, NBB, GMAX, chunk_off


def _build_program(g, CTOT, NBB, GMAX, IDXCOLS, chunk_off):
    import concourse.bass as bass
    import concourse.bacc as bacc
    import concourse.mybir as mybir
    import concourse.tile as tile
    from concourse.masks import make_identity

    f32, bf16 = mybir.dt.float32, mybir.dt.bfloat16
    i8, i16 = mybir.dt.int8, mybir.dt.int16
    Alu, Act = mybir.AluOpType, mybir.ActivationFunctionType
    F1 = F_IN
    F2 = 128
    n_lo, n_hi = g['n_lo_max'], g['n_hi_max']
    groups = g['groups']

    nc = bacc.Bacc(None, target_bir_lowering=False, debug=False, num_swdge_queues=4)

    t_xT = nc.dram_tensor("xT", [F_IN, NODES_PER_CORE], bf16, kind="ExternalInput")
    t_idx = nc.dram_tensor("idx", [P, IDXCOLS], i16, kind="ExternalInput")
    t_dstloc = nc.dram_tensor("dstloc", [P, CTOT], i8, kind="ExternalInput")
    t_dstlocT = nc.dram_tensor("dstlocT", [1, CTOT * P], i8, kind="ExternalInput")
    t_W1 = nc.dram_tensor("Wcat1", [F_IN, 264], bf16, kind="ExternalInput")
    t_W2 = nc.dram_tensor("Wcat2", [F_IN, 42], bf16, kind="ExternalInput")
    t_Rinv = nc.dram_tensor("RinvT", [P, 2, P], bf16, kind="ExternalInput")
    t_iotaR = nc.dram_tensor("iotaR", [P, NBB * P], i8, kind="ExternalInput")
    t_iotaC = nc.dram_tensor("iotaC", [P, NBB * P], i8, kind="ExternalInput")
    t_b1 = nc.dram_tensor("b1B", [P, F1], f32, kind="ExternalInput")
    t_b2 = nc.dram_tensor("b2B", [P, C2], f32, kind="ExternalInput")
    t_out = nc.dram_tensor("out2", [NODES_PER_CORE, C2], f32, kind="ExternalOutput")

    with tile.TileContext(nc) as tc:
        with tc.tile_pool(name="dram", bufs=1, space="DRAM") as dram, \
             tc.tile_pool(name="const", bufs=1) as cst, \
             tc.tile_pool(name="resid", bufs=1) as res, \
             tc.tile_pool(name="work", bufs=2) as wk, \
             tc.tile_pool(name="sml", bufs=3) as sml, \
             tc.tile_pool(name="gath", bufs=2) as gp, \
             tc.tile_pool(name="indp", bufs=2) as indp, \
             tc.tile_pool(name="ps_agg", bufs=3, space="PSUM") as ps_agg, \
             tc.tile_pool(name="ps_er", bufs=1, space="PSUM") as ps_er, \
             tc.tile_pool(name="ps_t", bufs=2, space="PSUM") as ps_t, \
             tc.tile_pool(name="ps_f2", bufs=1, space="PSUM") as ps_f2:

            T1_locals = []
            T2_locals = []
            for k, (a, b) in enumerate(AGRANGES):
                T1_locals.append(dram.tile([(b - a) * P, F1], bf16, name=f"T1_local{k}", tag=f"t1l{k}"))
                T2_locals.append(dram.tile([(b - a) * P, F2], bf16, name=f"T2_local{k}", tag=f"t2l{k}"))
            T1_full = dram.tile([NPAD, F1], bf16)
            T2_full = dram.tile([NPAD, F2], bf16)

            iotaR = cst.tile([P, NBB * P], i8)
            nc.sync.dma_start(iotaR[:], t_iotaR[:])
            iotaC = cst.tile([P, NBB * P], i8)
            nc.sync.dma_start(iotaC[:], t_iotaC[:])
            b1B = cst.tile([P, F1], f32)
            nc.sync.dma_start(b1B[:], t_b1[:])
            b2B = cst.tile([P, C2], f32)
            nc.sync.dma_start(b2B[:], t_b2[:])
            Wc2 = cst.tile([P, 2, 42], bf16)
            nc.sync.dma_start(Wc2[:, 0, :], t_W2[0:128, :])
            nc.sync.dma_start(Wc2[:, 1, :], t_W2[128:256, :])
            Rinv = cst.tile([P, 2, P], bf16)
            nc.sync.dma_start(Rinv[:], t_Rinv[:])
            identb = cst.tile([P, P], bf16)
            make_identity(nc, identb[:])
            alpha = cst.tile([P, 1], f32)
            nc.vector.memset(alpha[:], NEG_SLOPE)
            er1_sb = res.tile([P, BLOCKS_PER_CORE * H1], bf16)
            er2_sb = res.tile([P, BLOCKS_PER_CORE], bf16)
            idx_sb = res.tile([P, IDXCOLS], i16)
            nc.sync.dma_start(idx_sb[:], t_idx[:])
            dstloc_sb = res.tile([P, CTOT], i8)
            nc.sync.dma_start(dstloc_sb[:], t_dstloc[:])

            # ---- phase 1 (+ split AllGathers) ----
            with tc.tile_pool(name="p1", bufs=3) as p1, \
                 tc.tile_pool(name="p1w", bufs=1) as p1w:
                w1a = p1w.tile([P, 264], bf16)
                nc.sync.dma_start(w1a[:], t_W1[0:128, :])
                w1b = p1w.tile([P, 264], bf16)
                nc.sync.dma_start(w1b[:], t_W1[128:256, :])
                for k, (ra, rb) in enumerate(AGRANGES):
                    for i in range(ra, rb):
                        sl = slice(i * P, (i + 1) * P)
                        lsl = slice((i - ra) * P, (i - ra + 1) * P)
                        xt = p1.tile([P, 2, P], bf16, tag="xt")
                        nc.sync.dma_start(
                            xt[:], t_xT.rearrange("(a p) n -> p a n", a=2)[:, :, sl])
                        acc = ps_agg.tile([P, 264], f32, space="PSUM", tag="agg")
                        nc.tensor.matmul(acc[:], lhsT=xt[:, 0, :], rhs=w1a[:],
                                         start=True, stop=False)
                        nc.tensor.matmul(acc[:], lhsT=xt[:, 1, :], rhs=w1b[:],
                                         start=False, stop=True)
                        fb = p1.tile([P, F1], bf16, tag="p1out")
                        nc.scalar.copy(out=fb[:], in_=acc[:, 0:F1])
                        nc.sync.dma_start(T1_locals[k][lsl, :], fb[:])
                        nc.vector.tensor_copy(out=er1_sb[:, i * H1:(i + 1) * H1],
                                              in_=acc[:, 256:264])
                    nc.gpsimd.collective_compute(
                        "AllGather", mybir.AluOpType.bypass,
                        replica_groups=[list(range(NCORES))],
                        ins=[T1_locals[k][:]],
                        outs=[T1_full[AGR_TAB[k][0]:AGR_TAB[k][1], :]])

            qctr = [0]

            SUBCH = 12

            def gather_group(Gt, grp, table_full, elem, icol):
                col = 0
                for kind in ('lo', 'hi'):
                    tbl = (table_full[0:SPLIT, :] if kind == 'lo'
                           else table_full[SPLIT:NPAD, :])
                    nch = int(sum((n_lo if kind == 'lo' else n_hi)[i] for i in grp))
                    while nch > 0:
                        m = min(nch, SUBCH)
                        if nch - m == 1:
                            m = nch  # avoid a tiny 1-chunk tail
                        q = qctr[0] % 4
                        qctr[0] += 1
                        nc.gpsimd.dma_gather(
                            Gt[:, col:col + m, :], tbl,
                            idx_sb[:, icol:icol + m * 8], m * P, m * P, elem,
                            single_packet=False, queue_num=q)
                        icol += m * 8
                        col += m
                        nch -= m
                return icol

            def group_loc(grp):
                loc = {}
                col = 0
                for kind in ('lo', 'hi'):
                    for i in grp:
                        nch = int((n_lo if kind == 'lo' else n_hi)[i])
                        loc[(i, kind)] = (col, nch)
                        col += nch
                return loc

            def build_indicators(bm, nbi):
                ind = indp.tile([P, NBB * P], bf16, tag="ind")
                indT = indp.tile([P, NBB * P], bf16, tag="indT")
                dT = indp.tile([P, NBB * P], i8, tag="dT")
                nc.sync.dma_start(
                    dT[:, 0:nbi * P].rearrange("p (o e) -> p o e", o=1),
                    t_dstlocT[None, :, bm * P:(bm + nbi) * P]
                        .to_broadcast([P, 1, nbi * P]))
                nc.vector.tensor_tensor(
                    out=ind[:, 0:nbi * P].rearrange("p (a b) -> p a b", b=P),
                    in0=dstloc_sb[:, bm:bm + nbi, None].to_broadcast([P, nbi, P]),
                    in1=iotaR[:, 0:nbi * P].rearrange("p (a b) -> p a b", b=P),
                    op=Alu.is_equal)
                nc.vector.tensor_tensor(
                    out=indT[:, 0:nbi * P], in0=iotaC[:, 0:nbi * P],
                    in1=dT[:, 0:nbi * P], op=Alu.is_equal)
                return ind, indT

            # ---- layer 1 edge phase ----
            icol = 0
            for grp in groups:
                G = gp.tile([P, GMAX, F1], bf16, tag="g1")
                icol = gather_group(G, grp, T1_full, F1, icol)
                loc = group_loc(grp)
                for i in grp:
                    nbi = int(g['nb_tot'][i])
                    if nbi == 0:
                        continue
                    parts = [(loc[(i, k)][0], loc[(i, k)][1])
                             for k in ('lo', 'hi') if loc[(i, k)][1] > 0]
                    ind, indT = build_indicators(chunk_off[i], nbi)

                    ers = ps_er.tile([P, NBB * H1], f32, space="PSUM", tag="ers")
                    for cc in range(nbi):
                        nc.tensor.matmul(ers[:, cc * H1:(cc + 1) * H1],
                                         lhsT=indT[:, cc * P:(cc + 1) * P],
                                         rhs=er1_sb[:, i * H1:(i + 1) * H1],
                                         start=True, stop=True)
                    ee = sml.tile([P, NBB * H1], f32, tag="ee")
                    bc = 0
                    for (lc, n) in parts:
                        nc.vector.tensor_tensor(
                            out=ee[:, bc * H1:(bc + n) * H1]
                                .rearrange("p (a h) -> p a h", h=H1),
                            in0=G[:, lc:lc + n, :]
                                .rearrange("p a (h d) -> p a h d", d=HD)
                                [:, :, :, 31:32]
                                .rearrange("p a h d -> p a (h d)"),
                            in1=ers[:, bc * H1:(bc + n) * H1]
                                .rearrange("p (a h) -> p a h", h=H1),
                            op=Alu.add)
                        bc += n
                    nc.scalar.activation(ee[:, 0:nbi * H1], ee[:, 0:nbi * H1],
                                         Act.Prelu, alpha=alpha[:, :1])
                    w = sml.tile([P, NBB * H1], bf16, tag="w")
                    nc.scalar.activation(w[:, 0:nbi * H1], ee[:, 0:nbi * H1], Act.Exp)

                    rhs_all = wk.tile([P, NBB, 264], bf16, tag="rhsall")
                    bc = 0
                    for (lc, n) in parts:
                        nc.vector.tensor_tensor(
                            out=rhs_all[:, bc:bc + n, 0:F1]
                                .rearrange("p a (h d) -> p a h d", d=HD),
                            in0=G[:, lc:lc + n, :]
                                .rearrange("p a (h d) -> p a h d", d=HD),
                            in1=w[:, bc * H1:(bc + n) * H1]
                                .rearrange("p (a h) -> p a h", h=H1)[:, :, :, None]
                                .to_broadcast([P, n, H1, HD]),
                            op=Alu.mult)
                        bc += n
                    nc.scalar.copy(
                        out=rhs_all[:, 0:nbi, F1:264],
                        in_=w[:, 0:nbi * H1].rearrange("p (a h) -> p a h", h=H1))

                    acc = ps_agg.tile([P, 264], f32, space="PSUM", tag="agg")
                    for cc in range(nbi):
                        nc.tensor.matmul(acc[:], lhsT=ind[:, cc * P:(cc + 1) * P],
                                         rhs=rhs_all[:, cc, :],
                                         start=(cc == 0), stop=(cc == nbi - 1))

                    den = sml.tile([P, H1], f32, tag="den")
                    nc.vector.tensor_scalar_max(den[:], acc[:, F1:264], 1e-30)
                    rec = sml.tile([P, H1], f32, tag="rec")
                    nc.vector.reciprocal(rec[:], den[:])
                    y_sb = sml.tile([P, F1], bf16, tag="ysb")
                    nc.scalar.copy(out=y_sb[:], in_=acc[:, 0:F1])

                    rot = ps_f2.tile([P, F1], f32, space="PSUM", tag="rot")
                    for j in range(2):
                        yt_ps = ps_t.tile([P, P], bf16, space="PSUM", tag="ytp")
                        nc.tensor.transpose(yt_ps[:], y_sb[:, j * P:(j + 1) * P],
                                            identb[:])
                        yt = sml.tile([P, P], bf16, tag="yt")
                        nc.scalar.copy(out=yt[:], in_=yt_ps[:])
                        nc.tensor.matmul(rot[:, j * P:(j + 1) * P], lhsT=yt[:],
                                         rhs=Rinv[:, j, :], start=True, stop=True)

                    h = sml.tile([P, F1], f32, tag="h")
                    nc.vector.tensor_tensor(
                        out=h[:].rearrange("p (a d) -> p a d", d=HD),
                        in0=rot[:].rearrange("p (a d) -> p a d", d=HD),
                        in1=rec[:, :, None].to_broadcast([P, H1, HD]), op=Alu.mult)
                    nc.vector.tensor_tensor(out=h[:], in0=h[:], in1=b1B[:], op=Alu.add)
                    hb = sml.tile([P, F1], bf16, tag="hb")
                    nc.vector.tensor_scalar_max(hb[:], h[:], 0.0)

                    f2 = ps_f2.tile([P, 42], f32, space="PSUM", tag="f2")
                    for j in range(2):
                        ht_ps = ps_t.tile([P, P], bf16, space="PSUM", tag="ytp")
                        nc.tensor.transpose(ht_ps[:], hb[:, j * P:(j + 1) * P],
                                            identb[:])
                        ht = sml.tile([P, P], bf16, tag="ht")
                        nc.scalar.copy(out=ht[:], in_=ht_ps[:])
                        nc.tensor.matmul(f2[:], lhsT=ht[:], rhs=Wc2[:, j, :],
                                         start=(j == 0), stop=(j == 1))
                    t2r = sml.tile([P, F2], bf16, tag="t2r")
                    nc.scalar.copy(out=t2r[:, 0:C2], in_=f2[:, 0:C2])
                    nc.vector.memset(t2r[:, C2:C2 + 1], 1.0)
                    nc.vector.tensor_copy(out=t2r[:, C2 + 1:C2 + 2],
                                          in_=f2[:, C2:C2 + 1])
                    nc.vector.tensor_copy(out=er2_sb[:, i:i + 1], in_=f2[:, 41:42])
                    for k, (ra, rb) in enumerate(AGRANGES):
                        if ra <= i < rb:
                            nc.sync.dma_start(
                                T2_locals[k][(i - ra) * P:(i - ra + 1) * P, :],
                                t2r[:])

            for k in range(len(AGRANGES)):
                nc.gpsimd.collective_compute(
                    "AllGather", mybir.AluOpType.bypass,
                    replica_groups=[list(range(NCORES))],
                    ins=[T2_locals[k][:]],
                    outs=[T2_full[AGR_TAB[k][0]:AGR_TAB[k][1], :]])

            # ---- layer 2 edge phase ----
            icol = 0
            for grp in groups:
                G2 = gp.tile([P, GMAX, F2], bf16, tag="g2")
                icol = gather_group(G2, grp, T2_full, F2, icol)
                loc = group_loc(grp)
                for i in grp:
                    nbi = int(g['nb_tot'][i])
                    if nbi == 0:
                        continue
                    parts = [(loc[(i, k)][0], loc[(i, k)][1])
                             for k in ('lo', 'hi') if loc[(i, k)][1] > 0]
                    ind, indT = build_indicators(chunk_off[i], nbi)

                    ers = ps_er.tile([P, NBB * H1], f32, space="PSUM", tag="ers")
                    for cc in range(nbi):
                        nc.tensor.matmul(ers[:, cc:cc + 1],
                                         lhsT=indT[:, cc * P:(cc + 1) * P],
                                         rhs=er2_sb[:, i:i + 1],
                                         start=True, stop=True)
                    ee = sml.tile([P, NBB], f32, tag="ee2")
                    bc = 0
                    for (lc, n) in parts:
                        nc.vector.tensor_tensor(
                            out=ee[:, bc:bc + n],
                            in0=G2[:, lc:lc + n, C2 + 1:C2 + 2]
                                .rearrange("p a d -> p (a d)"),
                            in1=ers[:, bc:bc + n], op=Alu.add)
                        bc += n
                    nc.scalar.activation(ee[:, 0:nbi], ee[:, 0:nbi], Act.Prelu,
                                         alpha=alpha[:, :1])
                    w2 = sml.tile([P, NBB], bf16, tag="w2")
                    nc.scalar.activation(w2[:, 0:nbi], ee[:, 0:nbi], Act.Exp)

                    rhs2 = wk.tile([P, NBB, 41], bf16, tag="rhs2")
                    bc = 0
                    for (lc, n) in parts:
                        nc.vector.tensor_tensor(
                            out=rhs2[:, bc:bc + n, :],
                            in0=G2[:, lc:lc + n, 0:41],
                            in1=w2[:, bc:bc + n, None].to_broadcast([P, n, 41]),
                            op=Alu.mult)
                        bc += n

                    acc = ps_agg.tile([P, 264], f32, space="PSUM", tag="agg")
                    for cc in range(nbi):
                        nc.tensor.matmul(acc[:, 0:41],
                                         lhsT=ind[:, cc * P:(cc + 1) * P],
                                         rhs=rhs2[:, cc, :],
                                         start=(cc == 0), stop=(cc == nbi - 1))

                    den = sml.tile([P, 1], f32, tag="den2")
                    nc.vector.tensor_scalar_max(den[:], acc[:, C2:41], 1e-30)
                    rec = sml.tile([P, 1], f32, tag="rec2")
                    nc.vector.reciprocal(rec[:], den[:])
                    o = sml.tile([P, C2], f32, tag="o")
                    nc.vector.tensor_tensor(out=o[:], in0=acc[:, 0:C2],
                                            in1=rec[:, :1].to_broadcast([P, C2]),
                                            op=Alu.mult)
                    nc.vector.tensor_tensor(out=o[:], in0=o[:], in1=b2B[:],
                                            op=Alu.add)
                    nc.sync.dma_start(t_out[i * P:(i + 1) * P, :], o[:])

    nc.compile()
    return nc


def kernel(features, src, dst, W1, attn_l1, attn_r1, b1, W2, attn_l2, attn_r2, b2):
    from concourse import bass_utils

    features = np.asarray(features, np.float32)
    src = np.asarray(src)
    dst = np.asarray(dst)
    W1 = np.asarray(W1, np.float32)
    attn_l1 = np.asarray(attn_l1, np.float32)
    attn_r1 = np.asarray(attn_r1, np.float32)
    b1 = np.asarray(b1, np.float32)
    W2 = np.asarray(W2, np.float32)
    attn_l2 = np.asarray(attn_l2, np.float32)
    attn_r2 = np.asarray(attn_r2, np.float32)
    b2 = np.asarray(b2, np.float32)

    g = _prep_graph(src, dst)
    per_core, CTOT, NBB, GMAX, chunk_off = _build_core_inputs(
        g, features, W1, attn_l1, attn_r1, W2, attn_l2, attn_r2, b1, b2)

    IDXCOLS = per_core[0]['idx'].shape[1]
    nc = _build_program(g, CTOT, NBB, GMAX, IDXCOLS, chunk_off)

    in_maps = []
    for pc in per_core:
        in_maps.append({
            "xT": pc['xT'], "idx": pc['idx'], "dstloc": pc['dstloc'],
            "dstlocT": pc['dstlocT'], "Wcat1": pc['Wcat1'], "Wcat2": pc['Wcat2'],
            "RinvT": pc['RinvT'], "iotaR": pc['iotaR'], "iotaC": pc['iotaC'],
            "b1B": pc['b1B'], "b2B": pc['b2B'],
        })

    res = bass_utils.run_bass_kernel_spmd(
        nc, in_maps, core_ids=list(range(NCORES)),
        trace=bool(int(__import__('os').environ.get('KTRACE', '0'))))
    kernel.last_result = res

    out = np.zeros((N_NODES, C2), np.float32)
    for c in range(NCORES):
        oc = res.results[c]["out2"]
        for i in range(BLOCKS_PER_CORE):
            b = g['blocks_at'][c][i]
            lo = b * P
            hi = min(lo + P, N_NODES)
            if hi > lo:
                out[lo:hi] = oc[i * P: i * P + (hi - lo)]
    return out



# revision 7
# speedup vs baseline: 1.1762x; 1.1762x over previous
"""Trainium2 Bass kernel for a 2-layer GAT (DGL-style) on a random graph.

v4 strategy (8 NeuronCores, SPMD, dst-block sharding):
  - 392 dst blocks of 128 nodes, LPT-balanced 49/core.
  - Per-head rotation: W1rot = [W1 @ R | Vr1] so el1 is one coordinate of
    each head's 32; columns stored d-major within each 128-half
    (col = half*128 + d*4 + h_local) so the per-edge w*y multiply has a
    step-1 innermost AP on both operands (DVE 2x eligibility).
  - Edge->dst indicator matrices (ind: slot x dst, indT: dst x slot) are
    precomputed on host as fp8e4 and DMA-streamed per group (HWDGE);
    no on-device is_equal. Mixed fp8xbf16 matmuls do er broadcast and
    dst aggregation.
  - SPLIT=25600 aligned with AGRANGES=[(0,25),(25,49)]: lo-gathers only
    depend on AllGather range 0, which fires mid-phase-1; T2 AllGather
    range 0 fires after the first 9 edge groups (positions 0..24).
  - Phase 1 (bf16): y1|er1 = x @ [W1R | Vr1]; er1 kept resident.
  - Edge phase per group: 2+ dma_gathers (lo/hi); per block: er via
    indT matmuls, w = exp(leakyrelu(el+er)), one accumulating matmul per
    128-edge chunk: psum += Ind^T @ [w*y | w]. Rotate-back + normalize +
    bias + relu -> h.
  - T2 rows [y2(40) | 1.0 | el2 | pad] bf16 (256B); L2 analogous with w
    folded into the 41-wide rhs; denominator from the baked 1.0 column.
"""

import sys
sys.path.insert(0, '/opt/trn_rl_repo')

import numpy as np
import ml_dtypes

N_NODES = 50000
N_EDGES = 800000
F_IN = 256
H1, HD = 8, 32
C2 = 40
NEG_SLOPE = 0.2
NCORES = 8
P = 128
BLOCKS_PER_CORE = 49
NODES_PER_CORE = BLOCKS_PER_CORE * P      # 6272
NPAD = NCORES * NODES_PER_CORE            # 50176
NBLOCKS = NPAD // P                       # 392
AGRANGES = [(0, 25), (25, 49)]
SPLIT = 25 * P * NCORES                   # 25600 == end of AG range 0
NG1, NG2 = 9, 8                           # edge groups per position-half

AGR_TAB = []
_off = 0
for (_a, _b) in AGRANGES:
    size = (_b - _a) * P * NCORES
    AGR_TAB.append((_off, _off + size))
    _off += size
assert _off == NPAD

# within-half d-major permutation: col' = half*128 + d*4 + h_local
# maps to old col = (half*4 + h_local)*32 + d
_PERM = np.zeros(F_IN, np.int64)
for _c in range(F_IN):
    _j, _r = _c // 128, _c % 128
    _d, _hl = _r // 4, _r % 4
    _PERM[_c] = (_j * 4 + _hl) * HD + _d


def _prep_graph(src, dst):
    src = src.astype(np.int64)
    dst = dst.astype(np.int64)
    blk_of_edge = dst // P
    blk_counts = np.bincount(blk_of_edge, minlength=NBLOCKS)

    order = np.argsort(-blk_counts)
    core_of_blk = np.zeros(NBLOCKS, np.int64)
    loads = np.zeros(NCORES, np.int64)
    fills = np.zeros(NCORES, np.int64)
    for b in order:
        cands = np.where(fills < BLOCKS_PER_CORE)[0]
        c = cands[np.argmin(loads[cands])]
        core_of_blk[b] = c
        loads[c] += blk_counts[b]
        fills[c] += 1

    pos_of_blk = np.zeros(NBLOCKS, np.int64)
    blocks_at = np.zeros((NCORES, BLOCKS_PER_CORE), np.int64)
    for c in range(NCORES):
        mine = np.where(core_of_blk == c)[0]
        mine = mine[np.argsort(-blk_counts[mine])]
        blocks_at[c] = mine
        pos_of_blk[mine] = np.arange(BLOCKS_PER_CORE)

    # range-major table row-block of (core, pos) for split AllGathers
    rowblk_of_corepos = np.zeros((NCORES, BLOCKS_PER_CORE), np.int64)
    for c in range(NCORES):
        for p_ in range(BLOCKS_PER_CORE):
            for (a, b) in AGRANGES:
                if a <= p_ < b:
                    rowblk_of_corepos[c, p_] = a * NCORES + c * (b - a) + (p_ - a)
                    break

    node_ids = np.arange(NPAD)
    nb = node_ids // P
    pos_of_node = (rowblk_of_corepos[core_of_blk[nb], pos_of_blk[nb]] * P
                   + (node_ids % P))

    e_order = np.argsort(blk_of_edge, kind='stable')
    blk_starts = np.zeros(NBLOCKS + 1, np.int64)
    np.cumsum(blk_counts, out=blk_starts[1:])

    srcpos = pos_of_node[src]
    dstloc = (dst % P).astype(np.int64)

    n_lo = np.zeros((NCORES, BLOCKS_PER_CORE), np.int64)
    n_hi = np.zeros((NCORES, BLOCKS_PER_CORE), np.int64)
    blk_lo_edges = {}
    blk_hi_edges = {}
    for b in range(NBLOCKS):
        es = e_order[blk_starts[b]:blk_starts[b + 1]]
        sp = srcpos[es]
        lo = es[sp < SPLIT]
        hi = es[sp >= SPLIT]
        lo = lo[np.argsort(srcpos[lo], kind='stable')]
        hi = hi[np.argsort(srcpos[hi], kind='stable')]
        blk_lo_edges[b] = lo
        blk_hi_edges[b] = hi
        c, i = core_of_blk[b], pos_of_blk[b]
        n_lo[c, i] = -(-len(lo) // P) if len(lo) else 0
        n_hi[c, i] = -(-len(hi) // P) if len(hi) else 0
    n_lo_max = n_lo.max(axis=0)
    n_hi_max = n_hi.max(axis=0)
    nb_tot = n_lo_max + n_hi_max

    # LPT-balanced groups within each AG position-half, so the first NG1
    # groups cover exactly positions [0,25) (the T2 AG range-0 inputs).
    def lpt(lo_p, hi_p, nbins):
        bins = [[] for _ in range(nbins)]
        bw = np.zeros(nbins)
        for p_ in sorted(range(lo_p, hi_p), key=lambda x: -nb_tot[x]):
            k = int(bw.argmin())
            bins[k].append(p_)
            bw[k] += nb_tot[p_]
        return bins
    groups = lpt(0, 25, NG1) + lpt(25, 49, NG2)

    return dict(core_of_blk=core_of_blk, pos_of_blk=pos_of_blk, blocks_at=blocks_at,
                pos_of_node=pos_of_node, blk_lo=blk_lo_edges, blk_hi=blk_hi_edges,
                srcpos=srcpos, dstloc=dstloc, n_lo_max=n_lo_max, n_hi_max=n_hi_max,
                nb_tot=nb_tot, groups=groups)


def _pack_idx16(vals):
    v = np.asarray(vals, np.uint16).reshape(-1, 16).T
    return np.tile(v, (8, 1)).view(np.int16)


def _rotations(W1, attn_l1, attn_r1):
    """Per-head R_h = [Q_h | a_h] (Q orthonormal perp of a); W1rot = W1 @ R.
    Columns then permuted to within-half d-major order."""
    rng = np.random.default_rng(0)
    W1 = W1.astype(np.float64)
    W1rot = np.zeros((F_IN, F_IN), np.float64)
    Rinv_bd = np.zeros((F_IN, F_IN), np.float64)
    for h in range(H1):
        a = attn_l1[h].astype(np.float64)
        M = np.concatenate([a[:, None], rng.standard_normal((HD, HD - 1))], axis=1)
        Q, _ = np.linalg.qr(M)
        Qp = Q[:, 1:]
        R = np.concatenate([Qp, a[:, None]], axis=1)
        aa = float(a @ a)
        Rinv = np.concatenate([Qp.T, (a / aa)[None, :]], axis=0)
        assert np.allclose(R @ Rinv, np.eye(HD), atol=1e-9)
        sl = slice(h * HD, (h + 1) * HD)
        W1rot[:, sl] = W1[:, sl] @ R
        Rinv_bd[sl, sl] = Rinv
    Vr1 = np.einsum('khd,hd->kh', W1.reshape(F_IN, H1, HD), attn_r1.astype(np.float64))
    W1p = W1rot[:, _PERM]                          # [256, 256] permuted cols
    Wcat1 = np.concatenate([W1p, Vr1], axis=1)     # [256, 264]
    RinvP = Rinv_bd[_PERM][:, _PERM]               # permuted both ways
    return Wcat1.astype(np.float32), RinvP.astype(np.float32)


def _build_core_inputs(g, features, W1, attn_l1, attn_r1, W2, attn_l2, attn_r2,
                       b1, b2):
    bf = ml_dtypes.bfloat16
    f8 = ml_dtypes.float8_e4m3
    CTOT = int(g['nb_tot'].sum())
    NBB = int(g['nb_tot'].max())
    gch = [int(sum(g['nb_tot'][i] for i in grp)) for grp in g['groups']]
    GMAX = max(gch)

    Wcat1, RinvP = _rotations(W1, attn_l1, attn_r1)
    vl2 = W2 @ attn_l2[0]
    vr2 = W2 @ attn_r2[0]
    Wcat2 = np.concatenate([W2, vl2[:, None], vr2[:, None]], axis=1)[_PERM, :]

    b1B = np.tile(b1.astype(np.float32)[_PERM][None, :], (P, 1))
    b2B = np.tile(b2.astype(np.float32)[None, :], (P, 1))
    RinvT = np.zeros((P, 2, P), np.float32)
    RinvT[:, 0, :] = RinvP[0:128, 0:128]
    RinvT[:, 1, :] = RinvP[128:256, 128:256]
    # off-diagonal blocks of RinvP must be zero (heads stay within halves)
    assert np.abs(RinvP[0:128, 128:256]).max() == 0
    assert np.abs(RinvP[128:256, 0:128]).max() == 0

    feats_pad = np.zeros((NPAD, F_IN), np.float32)
    feats_pad[:N_NODES] = features

    per_core = []
    chunk_off = None
    for c in range(NCORES):
        my_nodes = (g['blocks_at'][c][:, None] * P + np.arange(P)[None, :]).reshape(-1)
        xT = feats_pad[my_nodes].T.astype(bf).copy()

        idx_cols = []
        # gather-order (group, kind, block) for idx packing
        for grp in g['groups']:
            for kind in ('lo', 'hi'):
                for i in grp:
                    b = g['blocks_at'][c][i]
                    nch = int((g['n_lo_max'] if kind == 'lo' else g['n_hi_max'])[i])
                    if nch == 0:
                        continue
                    es = g['blk_lo' if kind == 'lo' else 'blk_hi'][b]
                    sp = g['srcpos'][es] - (0 if kind == 'lo' else SPLIT)
                    npad_e = nch * P - len(es)
                    sp = np.concatenate([sp, np.zeros(npad_e, np.int64)])
                    idx_cols.append(_pack_idx16(sp))
        # block-major (group order) chunk layout for indicators
        dl_all = np.full((CTOT, P), -1, np.int64)
        ccol = 0
        co = {}
        for grp in g['groups']:
            for i in grp:
                co[i] = ccol
                b = g['blocks_at'][c][i]
                for kind in ('lo', 'hi'):
                    nch = int((g['n_lo_max'] if kind == 'lo' else g['n_hi_max'])[i])
                    if nch == 0:
                        continue
                    es = g['blk_lo' if kind == 'lo' else 'blk_hi'][b]
                    dl = g['dstloc'][es]
                    npad_e = nch * P - len(es)
                    dl = np.concatenate([dl, np.full(npad_e, -1, np.int64)])
                    dl_all[ccol:ccol + nch] = dl.reshape(nch, P)
                    ccol += nch
        assert ccol == CTOT, (ccol, CTOT)
        chunk_off = co
        idx_all = np.concatenate(idx_cols, axis=1)

        # indicators: ind[cc][slot, d] = (dl_all[cc, slot] == d)
        ar = np.arange(P)
        ind_b = (dl_all[:, :, None] == ar[None, None, :])        # [CTOT, slot, d]
        ind8 = np.ascontiguousarray(
            ind_b.transpose(1, 0, 2).reshape(P, CTOT * P)).astype(f8)
        indT8 = np.ascontiguousarray(
            ind_b.transpose(2, 0, 1).reshape(P, CTOT * P)).astype(f8)

        per_core.append(dict(xT=xT, idx=idx_all, ind=ind8, indT=indT8,
                             Wcat1=Wcat1.astype(bf), Wcat2=Wcat2.astype(bf),
                             RinvT=RinvT.astype(bf), b1B=b1B, b2B=b2B))
    return per_core, CTOT, NBB, GMAX, gch, chunk_off


def _build_program(g, CTOT, NBB, GMAX, gch, IDXCOLS, chunk_off, skip_b1, skip_b2):
    import concourse.bass as bass
    import concourse.bacc as bacc
    import concourse.mybir as mybir
    import concourse.tile as tile
    from concourse.masks import make_identity

    f32, bf16 = mybir.dt.float32, mybir.dt.bfloat16
    i16, fp8 = mybir.dt.int16, mybir.dt.float8e4
    Alu, Act = mybir.AluOpType, mybir.ActivationFunctionType
    F1 = F_IN
    F2 = 128
    n_lo, n_hi = g['n_lo_max'], g['n_hi_max']
    groups = g['groups']

    nc = bacc.Bacc(None, target_bir_lowering=False, debug=False, num_swdge_queues=4)

    t_xT = nc.dram_tensor("xT", [F_IN, NODES_PER_CORE], bf16, kind="ExternalInput")
    t_idx = nc.dram_tensor("idx", [P, IDXCOLS], i16, kind="ExternalInput")
    t_ind = nc.dram_tensor("ind", [P, CTOT * P], fp8, kind="ExternalInput")
    t_indT = nc.dram_tensor("indT", [P, CTOT * P], fp8, kind="ExternalInput")
    t_W1 = nc.dram_tensor("Wcat1", [F_IN, 264], bf16, kind="ExternalInput")
    t_W2 = nc.dram_tensor("Wcat2", [F_IN, 42], bf16, kind="ExternalInput")
    t_Rinv = nc.dram_tensor("RinvT", [P, 2, P], bf16, kind="ExternalInput")
    t_b1 = nc.dram_tensor("b1B", [P, F1], f32, kind="ExternalInput")
    t_b2 = nc.dram_tensor("b2B", [P, C2], f32, kind="ExternalInput")
    t_out = nc.dram_tensor("out2", [NODES_PER_CORE, C2], f32, kind="ExternalOutput")

    # group -> starting chunk col (block-major chunk layout is contiguous per
    # group because chunk_off was assigned in group order)
    g_start = []
    acc_ = 0
    for k in range(len(groups)):
        g_start.append(acc_)
        acc_ += gch[k]
    assert acc_ == CTOT

    with tile.TileContext(nc) as tc:
        with tc.tile_pool(name="dram", bufs=1, space="DRAM") as dram, \
             tc.tile_pool(name="const", bufs=1) as cst, \
             tc.tile_pool(name="resid", bufs=1) as res, \
             tc.tile_pool(name="work", bufs=2) as wk, \
             tc.tile_pool(name="sml", bufs=3) as sml, \
             tc.tile_pool(name="gath", bufs=2) as gp, \
             tc.tile_pool(name="indp", bufs=2) as indp, \
             tc.tile_pool(name="ps_agg", bufs=3, space="PSUM") as ps_agg, \
             tc.tile_pool(name="ps_er", bufs=1, space="PSUM") as ps_er, \
             tc.tile_pool(name="ps_t", bufs=2, space="PSUM") as ps_t, \
             tc.tile_pool(name="ps_f2", bufs=1, space="PSUM") as ps_f2:

            T1_locals = []
            T2_locals = []
            T1_fulls = []
            T2_fulls = []
            for k, (a, b) in enumerate(AGRANGES):
                T1_locals.append(dram.tile([(b - a) * P, F1], bf16, name=f"T1_local{k}", tag=f"t1l{k}"))
                T2_locals.append(dram.tile([(b - a) * P, F2], bf16, name=f"T2_local{k}", tag=f"t2l{k}"))
                T1_fulls.append(dram.tile([(b - a) * P * NCORES, F1], bf16, name=f"T1_full{k}",
                                          addr_space="Shared", tag=f"t1f{k}"))
                T2_fulls.append(dram.tile([(b - a) * P * NCORES, F2], bf16, name=f"T2_full{k}",
                                          addr_space="Shared", tag=f"t2f{k}"))

            b1B = cst.tile([P, F1], f32)
            nc.sync.dma_start(b1B[:], t_b1[:])
            b2B = cst.tile([P, C2], f32)
            nc.sync.dma_start(b2B[:], t_b2[:])
            Wc2 = cst.tile([P, 2, 42], bf16)
            nc.sync.dma_start(Wc2[:, 0, :], t_W2[0:128, :])
            nc.sync.dma_start(Wc2[:, 1, :], t_W2[128:256, :])
            Rinv = cst.tile([P, 2, P], bf16)
            nc.sync.dma_start(Rinv[:], t_Rinv[:])
            identb = cst.tile([P, P], bf16)
            make_identity(nc, identb[:])
            alpha = cst.tile([P, 1], f32)
            nc.vector.memset(alpha[:], NEG_SLOPE)
            er1_sb = res.tile([P, BLOCKS_PER_CORE * H1], bf16)
            er2_sb = res.tile([P, BLOCKS_PER_CORE], bf16)
            idx_sb = res.tile([P, IDXCOLS], i16)
            nc.sync.dma_start(idx_sb[:], t_idx[:])

            # ---- phase 1 (+ split AllGathers) ----
            with tc.tile_pool(name="p1", bufs=3) as p1, \
                 tc.tile_pool(name="p1w", bufs=1) as p1w:
                w1a = p1w.tile([P, 264], bf16)
                nc.sync.dma_start(w1a[:], t_W1[0:128, :])
                w1b = p1w.tile([P, 264], bf16)
                nc.sync.dma_start(w1b[:], t_W1[128:256, :])
                for k, (ra, rb) in enumerate(AGRANGES):
                    for i in range(ra, rb):
                        sl = slice(i * P, (i + 1) * P)
                        lsl = slice((i - ra) * P, (i - ra + 1) * P)
                        xt = p1.tile([P, 2, P], bf16, tag="xt")
                        nc.sync.dma_start(
                            xt[:], t_xT.rearrange("(a p) n -> p a n", a=2)[:, :, sl])
                        acc = ps_agg.tile([P, 264], f32, space="PSUM", tag="agg")
                        nc.tensor.matmul(acc[:], lhsT=xt[:, 0, :], rhs=w1a[:],
                                         start=True, stop=False)
                        nc.tensor.matmul(acc[:], lhsT=xt[:, 1, :], rhs=w1b[:],
                                         start=False, stop=True)
                        fb = p1.tile([P, F1], bf16, tag="p1out")
                        nc.scalar.copy(out=fb[:], in_=acc[:, 0:F1])
                        nc.sync.dma_start(T1_locals[k][lsl, :], fb[:])
                        nc.vector.tensor_copy(out=er1_sb[:, i * H1:(i + 1) * H1],
                                              in_=acc[:, 256:264])
                    nc.gpsimd.collective_compute(
                        "AllGather", mybir.AluOpType.bypass,
                        replica_groups=[list(range(NCORES))],
                        ins=[T1_locals[k][:]],
                        outs=[T1_fulls[k][:]])

            qctr = [0]

            SUBCH = 12

            def gather_group(Gt, grp, tables, elem, icol):
                col = 0
                for kind in ('lo', 'hi'):
                    tbl = tables[0][:] if kind == 'lo' else tables[1][:]
                    nch = int(sum((n_lo if kind == 'lo' else n_hi)[i] for i in grp))
                    while nch > 0:
                        m = min(nch, SUBCH)
                        if nch - m == 1:
                            m = nch  # avoid a tiny 1-chunk tail
                        q = qctr[0] % 4
                        qctr[0] += 1
                        nc.gpsimd.dma_gather(
                            Gt[:, col:col + m, :], tbl,
                            idx_sb[:, icol:icol + m * 8], m * P, m * P, elem,
                            single_packet=False, queue_num=q)
                        icol += m * 8
                        col += m
                        nch -= m
                return icol

            def group_loc(grp):
                loc = {}
                col = 0
                for kind in ('lo', 'hi'):
                    for i in grp:
                        nch = int((n_lo if kind == 'lo' else n_hi)[i])
                        loc[(i, kind)] = (col, nch)
                        col += nch
                return loc

            # ---- layer 1 edge phase ----
            icol = 0
            for gk, grp in enumerate(groups):
                G = gp.tile([P, GMAX, F1], bf16, tag="g1")
                icol = gather_group(G, grp, T1_fulls, F1, icol)
                ind_g = indp.tile([P, GMAX * P], fp8, tag="ind")
                nc.sync.dma_start(
                    ind_g[:, 0:gch[gk] * P],
                    t_ind[:, g_start[gk] * P:(g_start[gk] + gch[gk]) * P])
                indT_g = indp.tile([P, GMAX * P], fp8, tag="indT")
                nc.sync.dma_start(
                    indT_g[:, 0:gch[gk] * P],
                    t_indT[:, g_start[gk] * P:(g_start[gk] + gch[gk]) * P])
                loc = group_loc(grp)
                for i in grp:
                    nbi = int(g['nb_tot'][i])
                    if nbi == 0:
                        continue
                    parts = [(loc[(i, k)][0], loc[(i, k)][1])
                             for k in ('lo', 'hi') if loc[(i, k)][1] > 0]
                    ioff = (chunk_off[i] - g_start[gk]) * P

                    ers = ps_er.tile([P, NBB * H1], f32, space="PSUM", tag="ers")
                    for cc in range(nbi):
                        nc.tensor.matmul(ers[:, cc * H1:(cc + 1) * H1],
                                         lhsT=indT_g[:, ioff + cc * P:ioff + (cc + 1) * P],
                                         rhs=er1_sb[:, i * H1:(i + 1) * H1],
                                         start=True, stop=True)
                    ee = sml.tile([P, NBB * H1], f32, tag="ee")
                    bc = 0
                    for (lc, n) in parts:
                        nc.vector.tensor_tensor(
                            out=ee[:, bc * H1:(bc + n) * H1]
                                .rearrange("p (a j t) -> p a j t", j=2, t=4),
                            in0=G[:, lc:lc + n, :]
                                .rearrange("p a (j x) -> p a j x", j=2)
                                [:, :, :, 124:128],
                            in1=ers[:, bc * H1:(bc + n) * H1]
                                .rearrange("p (a j t) -> p a j t", j=2, t=4),
                            op=Alu.add)
                        bc += n
                    nc.scalar.activation(ee[:, 0:nbi * H1], ee[:, 0:nbi * H1],
                                         Act.Prelu, alpha=alpha[:, :1])
                    w = sml.tile([P, NBB * H1], bf16, tag="w")
                    nc.scalar.activation(w[:, 0:nbi * H1], ee[:, 0:nbi * H1], Act.Exp)

                    rhs_all = wk.tile([P, NBB, 264], bf16, tag="rhsall")
                    bc = 0
                    for (lc, n) in parts:
                        for j in range(2):
                            nc.vector.tensor_tensor(
                                out=rhs_all[:, bc:bc + n, j * 128:(j + 1) * 128]
                                    .rearrange("p a (d t) -> p a d t", t=4),
                                in0=G[:, lc:lc + n, j * 128:(j + 1) * 128]
                                    .rearrange("p a (d t) -> p a d t", t=4),
                                in1=w[:, bc * H1:(bc + n) * H1]
                                    .rearrange("p (a j t) -> p a j t", j=2, t=4)
                                    [:, :, j, None, :]
                                    .to_broadcast([P, n, HD, 4]),
                                op=Alu.mult)
                        bc += n
                    nc.scalar.copy(
                        out=rhs_all[:, 0:nbi, F1:264],
                        in_=w[:, 0:nbi * H1].rearrange("p (a h) -> p a h", h=H1))

                    acc = ps_agg.tile([P, 264], f32, space="PSUM", tag="agg")
                    for cc in range(nbi):
                        nc.tensor.matmul(acc[:],
                                         lhsT=ind_g[:, ioff + cc * P:ioff + (cc + 1) * P],
                                         rhs=rhs_all[:, cc, :],
                                         start=(cc == 0), stop=(cc == nbi - 1))

                    den = sml.tile([P, H1], f32, tag="den")
                    nc.vector.tensor_scalar_max(den[:], acc[:, F1:264], 1e-30)
                    rec = sml.tile([P, H1], f32, tag="rec")
                    nc.vector.reciprocal(rec[:], den[:])
                    y_sb = sml.tile([P, F1], bf16, tag="ysb")
                    nc.scalar.copy(out=y_sb[:], in_=acc[:, 0:F1])

                    rot = ps_f2.tile([P, F1], f32, space="PSUM", tag="rot")
                    for j in range(2):
                        yt_ps = ps_t.tile([P, P], bf16, space="PSUM", tag="ytp")
                        nc.tensor.transpose(yt_ps[:], y_sb[:, j * P:(j + 1) * P],
                                            identb[:])
                        yt = sml.tile([P, P], bf16, tag="yt")
                        nc.scalar.copy(out=yt[:], in_=yt_ps[:])
                        nc.tensor.matmul(rot[:, j * P:(j + 1) * P], lhsT=yt[:],
                                         rhs=Rinv[:, j, :], start=True, stop=True)

                    h = sml.tile([P, F1], f32, tag="h")
                    nc.vector.tensor_tensor(
                        out=h[:].rearrange("p (j d t) -> p j d t", j=2, t=4),
                        in0=rot[:].rearrange("p (j d t) -> p j d t", j=2, t=4),
                        in1=rec[:].rearrange("p (j t) -> p j t", j=2)
                            [:, :, None, :].to_broadcast([P, 2, HD, 4]),
                        op=Alu.mult)
                    if not skip_b1:
                        nc.vector.tensor_tensor(out=h[:], in0=h[:], in1=b1B[:],
                                                op=Alu.add)
                    hb = sml.tile([P, F1], bf16, tag="hb")
                    nc.vector.tensor_scalar_max(hb[:], h[:], 0.0)

                    f2 = ps_f2.tile([P, 42], f32, space="PSUM", tag="f2")
                    for j in range(2):
                        ht_ps = ps_t.tile([P, P], bf16, space="PSUM", tag="ytp")
                        nc.tensor.transpose(ht_ps[:], hb[:, j * P:(j + 1) * P],
                                            identb[:])
                        ht = sml.tile([P, P], bf16, tag="ht")
                        nc.scalar.copy(out=ht[:], in_=ht_ps[:])
                        nc.tensor.matmul(f2[:], lhsT=ht[:], rhs=Wc2[:, j, :],
                                         start=(j == 0), stop=(j == 1))
                    t2r = sml.tile([P, F2], bf16, tag="t2r")
                    nc.scalar.copy(out=t2r[:, 0:C2], in_=f2[:, 0:C2])
                    nc.vector.memset(t2r[:, C2:C2 + 1], 1.0)
                    nc.vector.tensor_copy(out=t2r[:, C2 + 1:C2 + 2],
                                          in_=f2[:, C2:C2 + 1])
                    nc.vector.tensor_copy(out=er2_sb[:, i:i + 1], in_=f2[:, 41:42])
                    for k, (ra, rb) in enumerate(AGRANGES):
                        if ra <= i < rb:
                            nc.sync.dma_start(
                                T2_locals[k][(i - ra) * P:(i - ra + 1) * P, :],
                                t2r[:])
                # fire the T2 AllGather for a position-half as soon as its
                # last group is done (groups 0..NG1-1 cover positions [0,25))
                if gk == NG1 - 1 or gk == len(groups) - 1:
                    k = 0 if gk == NG1 - 1 else 1
                    nc.gpsimd.collective_compute(
                        "AllGather", mybir.AluOpType.bypass,
                        replica_groups=[list(range(NCORES))],
                        ins=[T2_locals[k][:]],
                        outs=[T2_fulls[k][:]])

            # ---- layer 2 edge phase ----
            icol = 0
            for gk, grp in enumerate(groups):
                G2 = gp.tile([P, GMAX, F2], bf16, tag="g2")
                icol = gather_group(G2, grp, T2_fulls, F2, icol)
                ind_g = indp.tile([P, GMAX * P], fp8, tag="ind")
                nc.sync.dma_start(
                    ind_g[:, 0:gch[gk] * P],
                    t_ind[:, g_start[gk] * P:(g_start[gk] + gch[gk]) * P])
                indT_g = indp.tile([P, GMAX * P], fp8, tag="indT")
                nc.sync.dma_start(
                    indT_g[:, 0:gch[gk] * P],
                    t_indT[:, g_start[gk] * P:(g_start[gk] + gch[gk]) * P])
                loc = group_loc(grp)
                for i in grp:
                    nbi = int(g['nb_tot'][i])
                    if nbi == 0:
                        continue
                    parts = [(loc[(i, k)][0], loc[(i, k)][1])
                             for k in ('lo', 'hi') if loc[(i, k)][1] > 0]
                    ioff = (chunk_off[i] - g_start[gk]) * P

                    ers = ps_er.tile([P, NBB * H1], f32, space="PSUM", tag="ers")
                    for cc in range(nbi):
                        nc.tensor.matmul(ers[:, cc:cc + 1],
                                         lhsT=indT_g[:, ioff + cc * P:ioff + (cc + 1) * P],
                                         rhs=er2_sb[:, i:i + 1],
                                         start=True, stop=True)
                    ee = sml.tile([P, NBB], f32, tag="ee2")
                    bc = 0
                    for (lc, n) in parts:
                        nc.vector.tensor_tensor(
                            out=ee[:, bc:bc + n],
                            in0=G2[:, lc:lc + n, C2 + 1:C2 + 2]
                                .rearrange("p a d -> p (a d)"),
                            in1=ers[:, bc:bc + n], op=Alu.add)
                        bc += n
                    nc.scalar.activation(ee[:, 0:nbi], ee[:, 0:nbi], Act.Prelu,
                                         alpha=alpha[:, :1])
                    w2 = sml.tile([P, NBB], bf16, tag="w2")
                    nc.scalar.activation(w2[:, 0:nbi], ee[:, 0:nbi], Act.Exp)

                    rhs2 = wk.tile([P, NBB, 41], bf16, tag="rhs2")
                    bc = 0
                    for (lc, n) in parts:
                        nc.vector.tensor_tensor(
                            out=rhs2[:, bc:bc + n, :],
                            in0=G2[:, lc:lc + n, 0:41],
                            in1=w2[:, bc:bc + n, None].to_broadcast([P, n, 41]),
                            op=Alu.mult)
                        bc += n

                    acc = ps_agg.tile([P, 264], f32, space="PSUM", tag="agg")
                    for cc in range(nbi):
                        nc.tensor.matmul(acc[:, 0:41],
                                         lhsT=ind_g[:, ioff + cc * P:ioff + (cc + 1) * P],
                                         rhs=rhs2[:, cc, :],
                                         start=(cc == 0), stop=(cc == nbi - 1))

                    den = sml.tile([P, 1], f32, tag="den2")
                    nc.vector.tensor_scalar_max(den[:], acc[:, C2:41], 1e-30)
                    rec = sml.tile([P, 1], f32, tag="rec2")
                    nc.vector.reciprocal(rec[:], den[:])
                    o = sml.tile([P, C2], f32, tag="o")
                    nc.vector.tensor_tensor(out=o[:], in0=acc[:, 0:C2],
                                            in1=rec[:, :1].to_broadcast([P, C2]),
                                            op=Alu.mult)
                    if not skip_b2:
                        nc.vector.tensor_tensor(out=o[:], in0=o[:], in1=b2B[:],
                                                op=Alu.add)
                    nc.sync.dma_start(t_out[i * P:(i + 1) * P, :], o[:])

    nc.compile()
    return nc


def kernel(features, src, dst, W1, attn_l1, attn_r1, b1, W2, attn_l2, attn_r2, b2):
    from concourse import bass_utils

    features = np.asarray(features, np.float32)
    src = np.asarray(src)
    dst = np.asarray(dst)
    W1 = np.asarray(W1, np.float32)
    attn_l1 = np.asarray(attn_l1, np.float32)
    attn_r1 = np.asarray(attn_r1, np.float32)
    b1 = np.asarray(b1, np.float32)
    W2 = np.asarray(W2, np.float32)
    attn_l2 = np.asarray(attn_l2, np.float32)
    attn_r2 = np.asarray(attn_r2, np.float32)
    b2 = np.asarray(b2, np.float32)

    g = _prep_graph(src, dst)
    per_core, CTOT, NBB, GMAX, gch, chunk_off = _build_core_inputs(
        g, features, W1, attn_l1, attn_r1, W2, attn_l2, attn_r2, b1, b2)

    IDXCOLS = per_core[0]['idx'].shape[1]
    nc = _build_program(g, CTOT, NBB, GMAX, gch, IDXCOLS, chunk_off,
                        skip_b1=not b1.any(), skip_b2=not b2.any())

    in_maps = []
    for pc in per_core:
        in_maps.append({
            "xT": pc['xT'], "idx": pc['idx'], "ind": pc['ind'],
            "indT": pc['indT'], "Wcat1": pc['Wcat1'], "Wcat2": pc['Wcat2'],
            "RinvT": pc['RinvT'], "b1B": pc['b1B'], "b2B": pc['b2B'],
        })

    res = bass_utils.run_bass_kernel_spmd(
        nc, in_maps, core_ids=list(range(NCORES)),
        trace=bool(int(__import__('os').environ.get('KTRACE', '0'))))
    kernel.last_result = res

    out = np.zeros((N_NODES, C2), np.float32)
    for c in range(NCORES):
        oc = res.results[c]["out2"]
        for i in range(BLOCKS_PER_CORE):
            b = g['blocks_at'][c][i]
            lo = b * P
            hi = min(lo + P, N_NODES)
            if hi > lo:
                out[lo:hi] = oc[i * P: i * P + (hi - lo)]
    return out
